# revision 12
# baseline (speedup 1.0000x reference)
"""Trainium2 Bass kernel for the gated-attention multi-bag SSL head.

Computation (eval mode):
    H   = relu(x @ W1 + b1)                      [N, D]
    a   = (tanh(H@Wt+bt) * sigmoid(H@Ws+bs)) @ Wa + ba
    w   = segment_softmax(a, idxs)               (idxs sorted, 256 bags)
    M   = segment_sum(w * H)                     [B, D]
    proj= l2norm(M @ Wp + bp)                    [B, F]

Device strategy (8 NeuronCores, data-parallel over the instance dim N):
  * x is transposed on the host so the contraction dim L lands on SBUF
    partitions; each core gets a contiguous [L, N/8] shard streamed in
    ~8 MB DMAs that saturate HBM bandwidth (this kernel is memory-bound).
  * Softmax skips the segment-max: |a| <= F*max|Wa| ~ 6, so exp(a) is
    safe in fp32 and exp(a)/sum(exp(a)) == softmax(a).  This makes the
    whole kernel single-pass: each core accumulates U[b] = sum e_i*H_i
    and den[b] = sum e_i in one PSUM bank via one-hot matmuls.
  * Matmuls run in float32r (fp32 with an 11-bit mantissa) which streams
    at ~1 cycle/row instead of fp32's 4 — rel. error stays ~4e-5.
  * sigmoid(z) = 0.5*(1+tanh(z/2)); the 0.5 folds into Wa on the host.
    This keeps every activation (relu/tanh/exp) in ONE ACT table set.
  * Host combines per-core U/den (adjacent shards share at most one
    bag) and runs the tiny [256,128] projector epilogue.
"""

import numpy as np

N_CORES = 8
L, D, F, NBAGS = 1024, 128, 32, 256
N_TOTAL = 262144
SLICE = 512
UW = D + 1  # U output columns: 128 H-dims + 1 density column
UPAD = 256  # padded U-matmul width so float32r streams at 1 cyc/row
KCH = L // 128  # 8 contraction chunks

# float32r packed-constant layout (columns)
CR_W1 = 0  # [128, 1024]  W1 rearranged so chunk k is cols [128k, 128k+128)
CR_WT = 1024  # [128, 32]
CR_WS = 1056  # [128, 32]
CR_WA = 1088  # [32, 2]
CR_ID = 1090  # [128, 128] identity
CR_W = 1218
# float32 packed-constant layout (columns); idx width depends on n_rows
CF_IOTA = 0  # [128, 128]


def _chunk_plan(n_rows):
    """Full-rate 2048-col chunks with a tapered tail to shrink the
    after-last-DMA compute bubble."""
    assert n_rows % 2048 == 0
    chunks = [2048] * (n_rows // 2048 - 1) + [1024, 512, 512]
    assert sum(chunks) == n_rows and all(c % SLICE == 0 for c in chunks)
    return chunks

_CACHE = {}


def _build(n_rows):
    from contextlib import ExitStack

    import concourse.bacc as bacc
    import concourse.tile as tile
    from concourse import mybir

    F32 = mybir.dt.float32
    F32R = mybir.dt.float32r
    AF = mybir.ActivationFunctionType
    OP = mybir.AluOpType

    n_grp = n_rows // 128
    chunks = _chunk_plan(n_rows)
    n_u_mm = n_grp
    cf_w = 128 + n_grp + 4  # iota | idx | b1 | bt | bs | ba

    nc = bacc.Bacc(
        "TRN2", target_bir_lowering=False, debug=False, num_devices=N_CORES
    )
    xt = nc.dram_tensor("xt", [L, n_rows], F32R, kind="ExternalInput").ap()
    cr = nc.dram_tensor("cr", [128, CR_W], F32R, kind="ExternalInput").ap()
    cf = nc.dram_tensor("cf", [128, cf_w], F32, kind="ExternalInput").ap()
    u_out = nc.dram_tensor("u", [128, UW], F32, kind="ExternalOutput").ap()

    xt_r = xt.rearrange("(a p) n -> p a n", p=128)  # [128, KCH, n_rows]

    with tile.TileContext(nc) as tc, ExitStack() as ctx:
        const = ctx.enter_context(tc.tile_pool(name="const", bufs=1))
        xpool = ctx.enter_context(tc.tile_pool(name="xin", bufs=2))
        htp = ctx.enter_context(tc.tile_pool(name="htp", bufs=4))
        scp = ctx.enter_context(tc.tile_pool(name="scp", bufs=2))
        wop = ctx.enter_context(tc.tile_pool(name="wop", bufs=3))
        hnp = ctx.enter_context(tc.tile_pool(name="hnp", bufs=3))
        epool = ctx.enter_context(tc.tile_pool(name="ep", bufs=3))
        outp = ctx.enter_context(tc.tile_pool(name="outp", bufs=1))
        ph = ctx.enter_context(tc.tile_pool(name="ph", bufs=2, space="PSUM"))
        pts = ctx.enter_context(tc.tile_pool(name="pts", bufs=2, space="PSUM"))
        pa = ctx.enter_context(tc.tile_pool(name="pa", bufs=1, space="PSUM"))
        ptrp = ctx.enter_context(tc.tile_pool(name="ptrp", bufs=2, space="PSUM"))
        pu = ctx.enter_context(tc.tile_pool(name="pu", bufs=1, space="PSUM"))

        # ---- packed constants: two DMAs on the scalar HWDGE ring so the
        # sync ring starts streaming x immediately ----
        crsb = const.tile([128, CR_W], F32R, tag="cr")
        nc.scalar.dma_start(crsb[:], cr[:])
        cfsb = const.tile([128, cf_w], F32, tag="cf")
        nc.scalar.dma_start(cfsb[:], cf[:])

        w1sb = [crsb[:, CR_W1 + k * 128 : CR_W1 + (k + 1) * 128] for k in range(KCH)]
        wtsb = crsb[:, CR_WT : CR_WT + F]
        wssb = crsb[:, CR_WS : CR_WS + F]
        wasb = crsb[0:F, CR_WA : CR_WA + 2]
        identsb = crsb[:, CR_ID : CR_ID + 128]
        iotasb = cfsb[:, 0:128]
        idxsb = cfsb[:, 128 : 128 + n_grp]
        b1sb = cfsb[:, 128 + n_grp : 128 + n_grp + 1]
        btsb = cfsb[0:F, 128 + n_grp + 1 : 128 + n_grp + 2]
        bssb = cfsb[0:F, 128 + n_grp + 2 : 128 + n_grp + 3]
        basb = cfsb[:, 128 + n_grp + 3 : 128 + n_grp + 4]

        psum_u = pu.tile([128, UPAD], F32)
        state = {"u_mm": 0}

        def stage_reduce(ht, e, gcol_base):
            # transpose H back to natural layout, build weighted one-hot
            # lhsT, accumulate U/den.  Runs one slice behind stage_main so
            # the DVE/ACT producers stay ahead of the PE consumers.
            ptr_t = ptrp.tile([128, SLICE], F32R, tag="ptr")
            for j in range(SLICE // 128):
                nc.tensor.transpose(
                    ptr_t[:, j * 128 : (j + 1) * 128],
                    ht[:, j * 128 : (j + 1) * 128],
                    identsb,
                )
                hn = hnp.tile([128, UPAD], F32R, tag="hn")
                nc.vector.tensor_copy(
                    hn[:, 0:D], ptr_t[:, j * 128 : (j + 1) * 128]
                )
                # den column; cols D+1.. are never read out of PSUM
                nc.vector.memset(hn[:, D : D + 1].bitcast(F32), 1.0)
                wo = wop.tile([128, 128], F32R, tag="wo")
                nc.vector.tensor_scalar(
                    wo[:],
                    iotasb,
                    idxsb[:, gcol_base + j : gcol_base + j + 1],
                    e[:, 2 * j : 2 * j + 1],
                    op0=OP.is_equal,
                    op1=OP.mult,
                )
                nc.tensor.matmul(
                    psum_u[:],
                    wo[:],
                    hn[:],
                    start=(state["u_mm"] == 0),
                    stop=(state["u_mm"] == n_grp - 1),
                )
                state["u_mm"] += 1

        def stage_main(xk, c0):
            # H^T[d, n] accumulation over the 8 L-chunks
            psum_h = ph.tile([128, SLICE], F32, tag="psh")
            for k in range(KCH):
                nc.tensor.matmul(
                    psum_h[:],
                    w1sb[k],
                    xk[:, k, c0 : c0 + SLICE],
                    start=(k == 0),
                    stop=(k == KCH - 1),
                )
            ht = htp.tile([128, SLICE], F32R, tag="ht")
            nc.scalar.activation(ht[:], psum_h[:], AF.Relu, bias=b1sb)

            # gated attention scores (transposed layout [F, n])
            pt = pts.tile([F, SLICE], F32, tag="pts")
            ps = pts.tile([F, SLICE], F32, tag="pts")
            nc.tensor.matmul(pt[:], wtsb, ht[:], start=True, stop=True)
            nc.tensor.matmul(ps[:], wssb, ht[:], start=True, stop=True)
            at = scp.tile([F, SLICE], F32, tag="at")
            nc.scalar.activation(at[:], pt[:], AF.Tanh, bias=btsb)
            ts = scp.tile([F, SLICE], F32, tag="ts")
            nc.scalar.activation(ts[:], ps[:], AF.Tanh, bias=bssb, scale=0.5)
            g = scp.tile([F, SLICE], F32R, tag="g")
            nc.vector.scalar_tensor_tensor(
                g[:], ts[:], 1.0, at[:], op0=OP.add, op1=OP.mult
            )

            # a[n] as 128-row columns (even width for f32r), then exp
            pat = pa.tile([128, 8], F32, tag="pat")
            for j in range(SLICE // 128):
                nc.tensor.matmul(
                    pat[:, 2 * j : 2 * j + 2],
                    g[:, j * 128 : (j + 1) * 128],
                    wasb,
                    start=True,
                    stop=True,
                )
            e = epool.tile([128, 8], F32, tag="e")
            nc.scalar.activation(e[:], pat[:], AF.Exp, bias=basb)
            return ht, e

        pending = None
        col0 = 0
        for ch_cols in chunks:
            xk = xpool.tile([128, KCH, ch_cols], F32R, tag="xk")
            nc.sync.dma_start(xk[:], xt_r[:, :, col0 : col0 + ch_cols])
            for s in range(ch_cols // SLICE):
                ht, e = stage_main(xk, s * SLICE)
                if pending is not None:
                    stage_reduce(*pending)
                pending = (ht, e, col0 // 128 + s * (SLICE // 128))
            col0 += ch_cols
        stage_reduce(*pending)

        u_sb = outp.tile([128, UW], F32, tag="usb")
        nc.vector.tensor_copy(u_sb[:], psum_u[:, 0:UW])
        nc.sync.dma_start(u_out[:], u_sb[:])

    nc.compile()
    return nc


def _get_nc(n_rows):
    if n_rows not in _CACHE:
        _CACHE[n_rows] = _build(n_rows)
    return _CACHE[n_rows]


def _host_prep(x, idxs, W1, b1, Wt, bt, Ws, bs, Wa, ba):
    n_rows = x.shape[0] // N_CORES
    n_grp = n_rows // 128
    xT = np.ascontiguousarray(x.T)  # [L, N]

    cr = np.zeros((128, CR_W), np.float32)
    cr[:, CR_W1 : CR_W1 + L] = (
        W1.reshape(KCH, 128, D).transpose(1, 0, 2).reshape(128, L)
    )
    cr[:, CR_WT : CR_WT + F] = Wt
    cr[:, CR_WS : CR_WS + F] = Ws
    cr[0:F, CR_WA] = Wa.reshape(-1) * 0.5
    cr[0:F, CR_WA + 1] = 0.0
    cr[:, CR_ID : CR_ID + 128] = np.eye(128, dtype=np.float32)

    cf_w = 128 + n_grp + 4
    cf_shared = np.zeros((128, cf_w), np.float32)
    cf_shared[:, 0:128] = np.arange(128, dtype=np.float32)[None, :]
    cf_shared[:, 128 + n_grp] = b1.reshape(-1)
    cf_shared[0:F, 128 + n_grp + 1] = bt.reshape(-1)
    cf_shared[0:F, 128 + n_grp + 2] = 0.5 * bs.reshape(-1)
    cf_shared[:, 128 + n_grp + 3] = float(np.asarray(ba).reshape(-1)[0])

    in_maps = []
    bases = []
    for c in range(N_CORES):
        lo, hi = c * n_rows, (c + 1) * n_rows
        base = int(idxs[lo])
        span = int(idxs[hi - 1]) - base + 1
        if span > 128:
            return None, None  # triggers numpy fallback
        bases.append(base)
        idl = (idxs[lo:hi] - base).astype(np.float32)
        cf = cf_shared.copy()
        cf[:, 128 : 128 + n_grp] = idl.reshape(n_grp, 128).T
        in_maps.append(
            {"xt": np.ascontiguousarray(xT[:, lo:hi]), "cr": cr, "cf": cf}
        )
    return in_maps, bases


def _combine(results, bases, Wp, bp):
    U_full = np.zeros((NBAGS + 128, D), np.float64)
    den_full = np.zeros(NBAGS + 128, np.float64)
    for c in range(N_CORES):
        u = results[c]["u"]
        U_full[bases[c] : bases[c] + 128] += u[:, :D]
        den_full[bases[c] : bases[c] + 128] += u[:, D]
    U_full = U_full[:NBAGS]
    den_full = den_full[:NBAGS]
    den_safe = np.where(den_full == 0, 1.0, den_full)
    M = (U_full / den_safe[:, None]).astype(np.float32)
    proj = (M @ np.asarray(Wp, np.float32) + np.asarray(bp, np.float32)).astype(
        np.float32
    )
    nrm = np.maximum(np.linalg.norm(proj, axis=1, keepdims=True), 1e-12)
    proj = (proj / nrm).astype(np.float32)
    return M, proj


def _numpy_fallback(x, idxs, W1, b1, Wt, bt, Ws, bs, Wa, ba, Wp, bp):
    H = np.maximum(x @ W1 + b1, 0.0).astype(np.float32)
    At = np.tanh(H @ Wt + bt)
    As = 1.0 / (1.0 + np.exp(-(H @ Ws + bs)))
    a = ((At * As) @ Wa)[:, 0] + np.asarray(ba).reshape(-1)[0]
    a = a - a.max()
    e = np.exp(a)
    den = np.zeros(NBAGS)
    np.add.at(den, idxs, e)
    U = np.zeros((NBAGS, D))
    np.add.at(U, idxs, e[:, None] * H)
    den = np.where(den == 0, 1.0, den)
    M = (U / den[:, None]).astype(np.float32)
    proj = (M @ Wp + bp).astype(np.float32)
    nrm = np.maximum(np.linalg.norm(proj, axis=1, keepdims=True), 1e-12)
    return M, (proj / nrm).astype(np.float32)


def kernel(x, idxs, W1, b1, Wt, bt, Ws, bs, Wa, ba, Wp, bp):
    from concourse.bass_utils import run_bass_kernel_spmd

    x = np.ascontiguousarray(np.asarray(x), np.float32)
    idxs = np.asarray(idxs).astype(np.int64)
    args = [np.asarray(v, np.float32) for v in (W1, b1, Wt, bt, Ws, bs, Wa, ba)]
    W1, b1, Wt, bt, Ws, bs, Wa, ba = args
    Wp = np.asarray(Wp, np.float32)
    bp = np.asarray(bp, np.float32)

    in_maps, bases = _host_prep(x, idxs, W1, b1, Wt, bt, Ws, bs, Wa, ba)
    if in_maps is None:
        return _numpy_fallback(
            x, idxs, W1, b1, Wt, bt, Ws, bs, Wa, ba, Wp, bp
        )
    nc = _get_nc(x.shape[0] // N_CORES)
    res = run_bass_kernel_spmd(nc, in_maps, list(range(N_CORES)), trace=False)
    return _combine(res.results, bases, Wp, bp)


# revision 14
# speedup vs baseline: 1.0212x; 1.0212x over previous
"""Trainium2 Bass kernel for the gated-attention multi-bag SSL head.

Computation (eval mode):
    H   = relu(x @ W1 + b1)                      [N, D]
    a   = (tanh(H@Wt+bt) * sigmoid(H@Ws+bs)) @ Wa + ba
    w   = segment_softmax(a, idxs)               (idxs sorted, 256 bags)
    M   = segment_sum(w * H)                     [B, D]
    proj= l2norm(M @ Wp + bp)                    [B, F]

Device strategy (8 NeuronCores, data-parallel over the instance dim N):
  * x is transposed on the host so the contraction dim L lands on SBUF
    partitions; each core gets a contiguous [L, N/8] shard streamed in
    ~8 MB DMAs that saturate HBM bandwidth (this kernel is memory-bound).
  * Softmax skips the segment-max: |a| <= F*max|Wa| ~ 6, so exp(a) is
    safe in fp32 and exp(a)/sum(exp(a)) == softmax(a).  This makes the
    whole kernel single-pass: each core accumulates U[b] = sum e_i*H_i
    and den[b] = sum e_i in one PSUM bank via one-hot matmuls.
  * Matmuls run in float32r (fp32 with an 11-bit mantissa) which streams
    at ~1 cycle/row instead of fp32's 4 — rel. error stays ~4e-5.
  * sigmoid(z) = 0.5*(1+tanh(z/2)); the 0.5 folds into Wa on the host.
    This keeps every activation (relu/tanh/exp) in ONE ACT table set.
  * Host combines per-core U/den (adjacent shards share at most one
    bag) and runs the tiny [256,128] projector epilogue.
"""

import numpy as np

N_CORES = 8
L, D, F, NBAGS = 1024, 128, 32, 256
N_TOTAL = 262144
SLICE = 512
UW = D + 1  # U output columns: 128 H-dims + 1 density column
UPAD = 256  # padded U-matmul width so float32r streams at 1 cyc/row
KCH = L // 128  # 8 contraction chunks

# float32r packed-constant layout (columns)
CR_W1 = 0  # [128, 1024]  W1 rearranged so chunk k is cols [128k, 128k+128)
CR_WT = 1024  # [128, 32]
CR_WS = 1056  # [128, 32]
CR_WA = 1088  # [32, 2]
CR_ID = 1090  # [128, 128] identity
CR_W = 1218
# float32 packed-constant layout (columns); idx width depends on n_rows
CF_IOTA = 0  # [128, 128]


def _chunk_plan(n_rows):
    """Full-rate 2048-col chunks with a tapered tail to shrink the
    after-last-DMA compute bubble."""
    assert n_rows % 2048 == 0
    chunks = [2048] * (n_rows // 2048 - 1) + [1024, 512, 512]
    assert sum(chunks) == n_rows and all(c % SLICE == 0 for c in chunks)
    return chunks

_CACHE = {}


def _build(n_rows):
    from contextlib import ExitStack

    import concourse.bacc as bacc
    import concourse.tile as tile
    from concourse import mybir

    F32 = mybir.dt.float32
    F32R = mybir.dt.float32r
    AF = mybir.ActivationFunctionType
    OP = mybir.AluOpType

    n_grp = n_rows // 128
    chunks = _chunk_plan(n_rows)
    n_u_mm = n_grp
    cf_w = 128 + n_grp + 4  # iota | idx | b1 | bt | bs | ba

    nc = bacc.Bacc(
        "TRN2", target_bir_lowering=False, debug=False, num_devices=N_CORES
    )
    xt = nc.dram_tensor("xt", [L, n_rows], F32R, kind="ExternalInput").ap()
    cr = nc.dram_tensor("cr", [128, CR_W], F32R, kind="ExternalInput").ap()
    cf = nc.dram_tensor("cf", [128, cf_w], F32, kind="ExternalInput").ap()
    u_out = nc.dram_tensor("u", [128, UW], F32, kind="ExternalOutput").ap()

    xt_r = xt.rearrange("(a p) n -> p a n", p=128)  # [128, KCH, n_rows]

    with tile.TileContext(nc) as tc, ExitStack() as ctx:
        const = ctx.enter_context(tc.tile_pool(name="const", bufs=1))
        xpool = ctx.enter_context(tc.tile_pool(name="xin", bufs=2))
        htp = ctx.enter_context(tc.tile_pool(name="htp", bufs=5))
        scp = ctx.enter_context(tc.tile_pool(name="scp", bufs=3))
        wop = ctx.enter_context(tc.tile_pool(name="wop", bufs=3))
        hnp = ctx.enter_context(tc.tile_pool(name="hnp", bufs=3))
        epool = ctx.enter_context(tc.tile_pool(name="ep", bufs=4))
        outp = ctx.enter_context(tc.tile_pool(name="outp", bufs=1))
        ph = ctx.enter_context(tc.tile_pool(name="ph", bufs=2, space="PSUM"))
        pts = ctx.enter_context(tc.tile_pool(name="pts", bufs=2, space="PSUM"))
        pa = ctx.enter_context(tc.tile_pool(name="pa", bufs=1, space="PSUM"))
        ptrp = ctx.enter_context(tc.tile_pool(name="ptrp", bufs=2, space="PSUM"))
        pu = ctx.enter_context(tc.tile_pool(name="pu", bufs=1, space="PSUM"))

        # ---- packed constants: two DMAs on the scalar HWDGE ring so the
        # sync ring starts streaming x immediately ----
        crsb = const.tile([128, CR_W], F32R, tag="cr")
        nc.scalar.dma_start(crsb[:], cr[:])
        cfsb = const.tile([128, cf_w], F32, tag="cf")
        nc.scalar.dma_start(cfsb[:], cf[:])

        w1sb = [crsb[:, CR_W1 + k * 128 : CR_W1 + (k + 1) * 128] for k in range(KCH)]
        wtsb = crsb[:, CR_WT : CR_WT + F]
        wssb = crsb[:, CR_WS : CR_WS + F]
        wasb = crsb[0:F, CR_WA : CR_WA + 2]
        identsb = crsb[:, CR_ID : CR_ID + 128]
        iotasb = cfsb[:, 0:128]
        idxsb = cfsb[:, 128 : 128 + n_grp]
        b1sb = cfsb[:, 128 + n_grp : 128 + n_grp + 1]
        btsb = cfsb[0:F, 128 + n_grp + 1 : 128 + n_grp + 2]
        bssb = cfsb[0:F, 128 + n_grp + 2 : 128 + n_grp + 3]
        basb = cfsb[:, 128 + n_grp + 3 : 128 + n_grp + 4]

        psum_u = pu.tile([128, UPAD], F32)
        state = {"u_mm": 0}

        def stage_reduce(ht, e, gcol_base):
            # transpose H back to natural layout, build weighted one-hot
            # lhsT, accumulate U/den.  Runs one slice behind stage_main so
            # the DVE/ACT producers stay ahead of the PE consumers.
            ptr_t = ptrp.tile([128, SLICE], F32R, tag="ptr")
            for j in range(SLICE // 128):
                nc.tensor.transpose(
                    ptr_t[:, j * 128 : (j + 1) * 128],
                    ht[:, j * 128 : (j + 1) * 128],
                    identsb,
                )
                hn = hnp.tile([128, UPAD], F32R, tag="hn")
                nc.vector.tensor_copy(
                    hn[:, 0:D], ptr_t[:, j * 128 : (j + 1) * 128]
                )
                # den column; cols D+1.. are never read out of PSUM
                nc.vector.memset(hn[:, D : D + 1].bitcast(F32), 1.0)
                wo = wop.tile([128, 128], F32R, tag="wo")
                nc.vector.tensor_scalar(
                    wo[:],
                    iotasb,
                    idxsb[:, gcol_base + j : gcol_base + j + 1],
                    e[:, 2 * j : 2 * j + 1],
                    op0=OP.is_equal,
                    op1=OP.mult,
                )
                nc.tensor.matmul(
                    psum_u[:],
                    wo[:],
                    hn[:],
                    start=(state["u_mm"] == 0),
                    stop=(state["u_mm"] == n_grp - 1),
                )
                state["u_mm"] += 1

        def stage_h(xk, c0):
            # H^T[d, n] accumulation over the 8 L-chunks, then scores
            psum_h = ph.tile([128, SLICE], F32, tag="psh")
            for k in range(KCH):
                nc.tensor.matmul(
                    psum_h[:],
                    w1sb[k],
                    xk[:, k, c0 : c0 + SLICE],
                    start=(k == 0),
                    stop=(k == KCH - 1),
                )
            ht = htp.tile([128, SLICE], F32R, tag="ht")
            nc.scalar.activation(ht[:], psum_h[:], AF.Relu, bias=b1sb)

            # gated attention scores (transposed layout [F, n])
            pt = pts.tile([F, SLICE], F32, tag="pts")
            ps = pts.tile([F, SLICE], F32, tag="pts")
            nc.tensor.matmul(pt[:], wtsb, ht[:], start=True, stop=True)
            nc.tensor.matmul(ps[:], wssb, ht[:], start=True, stop=True)
            at = scp.tile([F, SLICE], F32, tag="at")
            nc.scalar.activation(at[:], pt[:], AF.Tanh, bias=btsb)
            ts = scp.tile([F, SLICE], F32, tag="ts")
            nc.scalar.activation(ts[:], ps[:], AF.Tanh, bias=bssb, scale=0.5)
            g = scp.tile([F, SLICE], F32R, tag="g")
            nc.vector.scalar_tensor_tensor(
                g[:], ts[:], 1.0, at[:], op0=OP.add, op1=OP.mult
            )
            return ht, g

        def stage_attn(ht, g, gcol_base):
            # a[n] as 128-row columns (even width for f32r), then exp
            pat = pa.tile([128, 8], F32, tag="pat")
            for j in range(SLICE // 128):
                nc.tensor.matmul(
                    pat[:, 2 * j : 2 * j + 2],
                    g[:, j * 128 : (j + 1) * 128],
                    wasb,
                    start=True,
                    stop=True,
                )
            e = epool.tile([128, 8], F32, tag="e")
            nc.scalar.activation(e[:], pat[:], AF.Exp, bias=basb)
            return ht, e, gcol_base

        pend_attn = None
        pend_red = None
        col0 = 0
        for ch_cols in chunks:
            xk = xpool.tile([128, KCH, ch_cols], F32R, tag="xk")
            nc.sync.dma_start(xk[:], xt_r[:, :, col0 : col0 + ch_cols])
            for s in range(ch_cols // SLICE):
                ht, g = stage_h(xk, s * SLICE)
                if pend_attn is not None:
                    new_red = stage_attn(*pend_attn)
                    if pend_red is not None:
                        stage_reduce(*pend_red)
                    pend_red = new_red
                pend_attn = (ht, g, col0 // 128 + s * (SLICE // 128))
            col0 += ch_cols
        new_red = stage_attn(*pend_attn)
        if pend_red is not None:
            stage_reduce(*pend_red)
        stage_reduce(*new_red)

        u_sb = outp.tile([128, UW], F32, tag="usb")
        nc.vector.tensor_copy(u_sb[:], psum_u[:, 0:UW])
        nc.sync.dma_start(u_out[:], u_sb[:])

    nc.compile()
    return nc


def _get_nc(n_rows):
    if n_rows not in _CACHE:
        _CACHE[n_rows] = _build(n_rows)
    return _CACHE[n_rows]


def _host_prep(x, idxs, W1, b1, Wt, bt, Ws, bs, Wa, ba):
    n_rows = x.shape[0] // N_CORES
    n_grp = n_rows // 128
    xT = np.ascontiguousarray(x.T)  # [L, N]

    cr = np.zeros((128, CR_W), np.float32)
    cr[:, CR_W1 : CR_W1 + L] = (
        W1.reshape(KCH, 128, D).transpose(1, 0, 2).reshape(128, L)
    )
    cr[:, CR_WT : CR_WT + F] = Wt
    cr[:, CR_WS : CR_WS + F] = Ws
    cr[0:F, CR_WA] = Wa.reshape(-1) * 0.5
    cr[0:F, CR_WA + 1] = 0.0
    cr[:, CR_ID : CR_ID + 128] = np.eye(128, dtype=np.float32)

    cf_w = 128 + n_grp + 4
    cf_shared = np.zeros((128, cf_w), np.float32)
    cf_shared[:, 0:128] = np.arange(128, dtype=np.float32)[None, :]
    cf_shared[:, 128 + n_grp] = b1.reshape(-1)
    cf_shared[0:F, 128 + n_grp + 1] = bt.reshape(-1)
    cf_shared[0:F, 128 + n_grp + 2] = 0.5 * bs.reshape(-1)
    cf_shared[:, 128 + n_grp + 3] = float(np.asarray(ba).reshape(-1)[0])

    in_maps = []
    bases = []
    for c in range(N_CORES):
        lo, hi = c * n_rows, (c + 1) * n_rows
        base = int(idxs[lo])
        span = int(idxs[hi - 1]) - base + 1
        if span > 128:
            return None, None  # triggers numpy fallback
        bases.append(base)
        idl = (idxs[lo:hi] - base).astype(np.float32)
        cf = cf_shared.copy()
        cf[:, 128 : 128 + n_grp] = idl.reshape(n_grp, 128).T
        in_maps.append(
            {"xt": np.ascontiguousarray(xT[:, lo:hi]), "cr": cr, "cf": cf}
        )
    return in_maps, bases


def _combine(results, bases, Wp, bp):
    U_full = np.zeros((NBAGS + 128, D), np.float64)
    den_full = np.zeros(NBAGS + 128, np.float64)
    for c in range(N_CORES):
        u = results[c]["u"]
        U_full[bases[c] : bases[c] + 128] += u[:, :D]
        den_full[bases[c] : bases[c] + 128] += u[:, D]
    U_full = U_full[:NBAGS]
    den_full = den_full[:NBAGS]
    den_safe = np.where(den_full == 0, 1.0, den_full)
    M = (U_full / den_safe[:, None]).astype(np.float32)
    proj = (M @ np.asarray(Wp, np.float32) + np.asarray(bp, np.float32)).astype(
        np.float32
    )
    nrm = np.maximum(np.linalg.norm(proj, axis=1, keepdims=True), 1e-12)
    proj = (proj / nrm).astype(np.float32)
    return M, proj


def _numpy_fallback(x, idxs, W1, b1, Wt, bt, Ws, bs, Wa, ba, Wp, bp):
    H = np.maximum(x @ W1 + b1, 0.0).astype(np.float32)
    At = np.tanh(H @ Wt + bt)
    As = 1.0 / (1.0 + np.exp(-(H @ Ws + bs)))
    a = ((At * As) @ Wa)[:, 0] + np.asarray(ba).reshape(-1)[0]
    a = a - a.max()
    e = np.exp(a)
    den = np.zeros(NBAGS)
    np.add.at(den, idxs, e)
    U = np.zeros((NBAGS, D))
    np.add.at(U, idxs, e[:, None] * H)
    den = np.where(den == 0, 1.0, den)
    M = (U / den[:, None]).astype(np.float32)
    proj = (M @ Wp + bp).astype(np.float32)
    nrm = np.maximum(np.linalg.norm(proj, axis=1, keepdims=True), 1e-12)
    return M, (proj / nrm).astype(np.float32)


def kernel(x, idxs, W1, b1, Wt, bt, Ws, bs, Wa, ba, Wp, bp):
    from concourse.bass_utils import run_bass_kernel_spmd

    x = np.ascontiguousarray(np.asarray(x), np.float32)
    idxs = np.asarray(idxs).astype(np.int64)
    args = [np.asarray(v, np.float32) for v in (W1, b1, Wt, bt, Ws, bs, Wa, ba)]
    W1, b1, Wt, bt, Ws, bs, Wa, ba = args
    Wp = np.asarray(Wp, np.float32)
    bp = np.asarray(bp, np.float32)

    in_maps, bases = _host_prep(x, idxs, W1, b1, Wt, bt, Ws, bs, Wa, ba)
    if in_maps is None:
        return _numpy_fallback(
            x, idxs, W1, b1, Wt, bt, Ws, bs, Wa, ba, Wp, bp
        )
    nc = _get_nc(x.shape[0] // N_CORES)
    res = run_bass_kernel_spmd(nc, in_maps, list(range(N_CORES)), trace=False)
    return _combine(res.results, bases, Wp, bp)


# revision 15
# speedup vs baseline: 1.0667x; 1.0446x over previous
"""Trainium2 Bass kernel for the gated-attention multi-bag SSL head.

Computation (eval mode):
    H   = relu(x @ W1 + b1)                      [N, D]
    a   = (tanh(H@Wt+bt) * sigmoid(H@Ws+bs)) @ Wa + ba
    w   = segment_softmax(a, idxs)               (idxs sorted, 256 bags)
    M   = segment_sum(w * H)                     [B, D]
    proj= l2norm(M @ Wp + bp)                    [B, F]

Device strategy (8 NeuronCores, data-parallel over the instance dim N):
  * x is transposed on the host so the contraction dim L lands on SBUF
    partitions; each core gets a contiguous [L, N/8] shard streamed in
    ~8 MB DMAs that saturate HBM bandwidth (this kernel is memory-bound).
  * Softmax skips the segment-max: |a| <= F*max|Wa| ~ 6, so exp(a) is
    safe in fp32 and exp(a)/sum(exp(a)) == softmax(a).  This makes the
    whole kernel single-pass: each core accumulates U[b] = sum e_i*H_i
    and den[b] = sum e_i in one PSUM bank via one-hot matmuls.
  * Matmuls run in float32r (fp32 with an 11-bit mantissa) which streams
    at ~1 cycle/row instead of fp32's 4 — rel. error stays ~4e-5.
  * sigmoid(z) = 0.5*(1+tanh(z/2)); the 0.5 folds into Wa on the host.
    This keeps every activation (relu/tanh/exp) in ONE ACT table set.
  * Host combines per-core U/den (adjacent shards share at most one
    bag) and runs the tiny [256,128] projector epilogue.
"""

import numpy as np

N_CORES = 8
L, D, F, NBAGS = 1024, 128, 32, 256
N_TOTAL = 262144
SLICE = 512
UW = D + 1  # U output columns: 128 H-dims + 1 density column
UPAD = 256  # padded U-matmul width so float32r streams at 1 cyc/row
KCH = L // 128  # 8 contraction chunks

# float32r packed-constant layout (columns)
CR_W1 = 0  # [128, 1024]  W1 rearranged so chunk k is cols [128k, 128k+128)
CR_WT = 1024  # [128, 32]
CR_WS = 1056  # [128, 32]
CR_WA = 1088  # [32, 2]
CR_ID = 1090  # [128, 128] identity
CR_W = 1218
# float32 packed-constant layout (columns); idx width depends on n_rows
CF_IOTA = 0  # [128, 128]


def _chunk_plan(n_rows):
    """Full-rate 2048-col chunks with a tapered tail to shrink the
    after-last-DMA compute bubble."""
    assert n_rows % 2048 == 0
    chunks = [2048] * (n_rows // 2048 - 1) + [1024, 512, 512]
    assert sum(chunks) == n_rows and all(c % SLICE == 0 for c in chunks)
    return chunks

_CACHE = {}


def _build(n_rows):
    from contextlib import ExitStack

    import concourse.bacc as bacc
    import concourse.tile as tile
    from concourse import mybir

    F32 = mybir.dt.float32
    F32R = mybir.dt.float32r
    AF = mybir.ActivationFunctionType
    OP = mybir.AluOpType

    n_grp = n_rows // 128
    chunks = _chunk_plan(n_rows)
    n_u_mm = n_grp
    cf_w = 128 + n_grp + 4  # iota | idx | b1 | bt | bs | ba

    nc = bacc.Bacc(
        "TRN2", target_bir_lowering=False, debug=False, num_devices=N_CORES
    )
    xt = nc.dram_tensor("xt", [L, n_rows], F32R, kind="ExternalInput").ap()
    cr = nc.dram_tensor("cr", [128, CR_W], F32R, kind="ExternalInput").ap()
    cf = nc.dram_tensor("cf", [128, cf_w], F32, kind="ExternalInput").ap()
    u_out = nc.dram_tensor("u", [128, UW], F32, kind="ExternalOutput").ap()

    xt_r = xt.rearrange("(a p) n -> p a n", p=128)  # [128, KCH, n_rows]

    with tile.TileContext(nc) as tc, ExitStack() as ctx:
        const = ctx.enter_context(tc.tile_pool(name="const", bufs=1))
        xpool = ctx.enter_context(tc.tile_pool(name="xin", bufs=2))
        htp = ctx.enter_context(tc.tile_pool(name="htp", bufs=5))
        scp = ctx.enter_context(tc.tile_pool(name="scp", bufs=3))
        wop = ctx.enter_context(tc.tile_pool(name="wop", bufs=3))
        hnp = ctx.enter_context(tc.tile_pool(name="hnp", bufs=3))
        epool = ctx.enter_context(tc.tile_pool(name="ep", bufs=4))
        outp = ctx.enter_context(tc.tile_pool(name="outp", bufs=1))
        ph = ctx.enter_context(tc.tile_pool(name="ph", bufs=2, space="PSUM"))
        pts = ctx.enter_context(tc.tile_pool(name="pts", bufs=2, space="PSUM"))
        pa = ctx.enter_context(tc.tile_pool(name="pa", bufs=1, space="PSUM"))
        ptrp = ctx.enter_context(tc.tile_pool(name="ptrp", bufs=2, space="PSUM"))
        pu = ctx.enter_context(tc.tile_pool(name="pu", bufs=1, space="PSUM"))

        # ---- packed constants: two DMAs on the scalar HWDGE ring so the
        # sync ring starts streaming x immediately ----
        crsb = const.tile([128, CR_W], F32R, tag="cr")
        nc.scalar.dma_start(crsb[:], cr[:])
        cfsb = const.tile([128, cf_w], F32, tag="cf")
        nc.scalar.dma_start(cfsb[:], cf[:])

        w1sb = [crsb[:, CR_W1 + k * 128 : CR_W1 + (k + 1) * 128] for k in range(KCH)]
        wtsb = crsb[:, CR_WT : CR_WT + F]
        wssb = crsb[:, CR_WS : CR_WS + F]
        wasb = crsb[0:F, CR_WA : CR_WA + 2]
        identsb = crsb[:, CR_ID : CR_ID + 128]
        iotasb = cfsb[:, 0:128]
        idxsb = cfsb[:, 128 : 128 + n_grp]
        b1sb = cfsb[:, 128 + n_grp : 128 + n_grp + 1]
        btsb = cfsb[0:F, 128 + n_grp + 1 : 128 + n_grp + 2]
        bssb = cfsb[0:F, 128 + n_grp + 2 : 128 + n_grp + 3]
        basb = cfsb[:, 128 + n_grp + 3 : 128 + n_grp + 4]

        psum_u = pu.tile([128, UPAD], F32)
        state = {"u_mm": 0}

        def stage_reduce(ht, e, gcol_base):
            # transpose H back to natural layout, build weighted one-hot
            # lhsT, accumulate U/den.  Runs one slice behind stage_main so
            # the DVE/ACT producers stay ahead of the PE consumers.
            ptr_t = ptrp.tile([128, SLICE], F32R, tag="ptr")
            for j in range(SLICE // 128):
                nc.tensor.transpose(
                    ptr_t[:, j * 128 : (j + 1) * 128],
                    ht[:, j * 128 : (j + 1) * 128],
                    identsb,
                )
                hn = hnp.tile([128, UPAD], F32R, tag="hn")
                nc.vector.tensor_copy(
                    hn[:, 0:D], ptr_t[:, j * 128 : (j + 1) * 128]
                )
                # den column; cols D+1.. are never read out of PSUM
                nc.vector.memset(hn[:, D : D + 1].bitcast(F32), 1.0)
                wo = wop.tile([128, 128], F32R, tag="wo")
                nc.vector.tensor_scalar(
                    wo[:],
                    iotasb,
                    idxsb[:, gcol_base + j : gcol_base + j + 1],
                    e[:, 2 * j : 2 * j + 1],
                    op0=OP.is_equal,
                    op1=OP.mult,
                )
                nc.tensor.matmul(
                    psum_u[:],
                    wo[:],
                    hn[:],
                    start=(state["u_mm"] == 0),
                    stop=(state["u_mm"] == n_grp - 1),
                )
                state["u_mm"] += 1

        def stage_h(xk, c0):
            # H^T[d, n] accumulation over the 8 L-chunks, then scores
            psum_h = ph.tile([128, SLICE], F32, tag="psh")
            for k in range(KCH):
                nc.tensor.matmul(
                    psum_h[:],
                    w1sb[k],
                    xk[:, k, c0 : c0 + SLICE],
                    start=(k == 0),
                    stop=(k == KCH - 1),
                )
            ht = htp.tile([128, SLICE], F32R, tag="ht")
            nc.scalar.activation(ht[:], psum_h[:], AF.Relu, bias=b1sb)

            # gated attention scores (transposed layout [F, n])
            pt = pts.tile([F, SLICE], F32, tag="pts")
            ps = pts.tile([F, SLICE], F32, tag="pts")
            nc.tensor.matmul(pt[:], wtsb, ht[:], start=True, stop=True)
            nc.tensor.matmul(ps[:], wssb, ht[:], start=True, stop=True)
            at = scp.tile([F, SLICE], F32, tag="at")
            nc.scalar.activation(at[:], pt[:], AF.Tanh, bias=btsb)
            ts = scp.tile([F, SLICE], F32, tag="ts")
            nc.scalar.activation(ts[:], ps[:], AF.Tanh, bias=bssb, scale=0.5)
            g = scp.tile([F, SLICE], F32R, tag="g")
            nc.vector.scalar_tensor_tensor(
                g[:], ts[:], 1.0, at[:], op0=OP.add, op1=OP.mult
            )
            return ht, g

        def stage_attn(ht, g, gcol_base):
            # a[n] as 128-row columns (even width for f32r), then exp
            pat = pa.tile([128, 8], F32, tag="pat")
            for j in range(SLICE // 128):
                nc.tensor.matmul(
                    pat[:, 2 * j : 2 * j + 2],
                    g[:, j * 128 : (j + 1) * 128],
                    wasb,
                    start=True,
                    stop=True,
                )
            e = epool.tile([128, 8], F32, tag="e")
            nc.scalar.activation(e[:], pat[:], AF.Exp, bias=basb)
            return ht, e, gcol_base

        pending = None
        col0 = 0
        for ch_cols in chunks:
            xk = xpool.tile([128, KCH, ch_cols], F32R, tag="xk")
            nc.sync.dma_start(xk[:], xt_r[:, :, col0 : col0 + ch_cols])
            for s in range(ch_cols // SLICE):
                ht, g = stage_h(xk, s * SLICE)
                new_red = stage_attn(ht, g, col0 // 128 + s * (SLICE // 128))
                if pending is not None:
                    stage_reduce(*pending)
                pending = new_red
            col0 += ch_cols
        stage_reduce(*pending)

        u_sb = outp.tile([128, UW], F32, tag="usb")
        nc.vector.tensor_copy(u_sb[:], psum_u[:, 0:UW])
        nc.sync.dma_start(u_out[:], u_sb[:])

    nc.compile()
    return nc


def _get_nc(n_rows):
    if n_rows not in _CACHE:
        _CACHE[n_rows] = _build(n_rows)
    return _CACHE[n_rows]


def _host_prep(x, idxs, W1, b1, Wt, bt, Ws, bs, Wa, ba):
    n_rows = x.shape[0] // N_CORES
    n_grp = n_rows // 128
    xT = np.ascontiguousarray(x.T)  # [L, N]

    cr = np.zeros((128, CR_W), np.float32)
    cr[:, CR_W1 : CR_W1 + L] = (
        W1.reshape(KCH, 128, D).transpose(1, 0, 2).reshape(128, L)
    )
    cr[:, CR_WT : CR_WT + F] = Wt
    cr[:, CR_WS : CR_WS + F] = Ws
    cr[0:F, CR_WA] = Wa.reshape(-1) * 0.5
    cr[0:F, CR_WA + 1] = 0.0
    cr[:, CR_ID : CR_ID + 128] = np.eye(128, dtype=np.float32)

    cf_w = 128 + n_grp + 4
    cf_shared = np.zeros((128, cf_w), np.float32)
    cf_shared[:, 0:128] = np.arange(128, dtype=np.float32)[None, :]
    cf_shared[:, 128 + n_grp] = b1.reshape(-1)
    cf_shared[0:F, 128 + n_grp + 1] = bt.reshape(-1)
    cf_shared[0:F, 128 + n_grp + 2] = 0.5 * bs.reshape(-1)
    cf_shared[:, 128 + n_grp + 3] = float(np.asarray(ba).reshape(-1)[0])

    in_maps = []
    bases = []
    for c in range(N_CORES):
        lo, hi = c * n_rows, (c + 1) * n_rows
        base = int(idxs[lo])
        span = int(idxs[hi - 1]) - base + 1
        if span > 128:
            return None, None  # triggers numpy fallback
        bases.append(base)
        idl = (idxs[lo:hi] - base).astype(np.float32)
        cf = cf_shared.copy()
        cf[:, 128 : 128 + n_grp] = idl.reshape(n_grp, 128).T
        in_maps.append(
            {"xt": np.ascontiguousarray(xT[:, lo:hi]), "cr": cr, "cf": cf}
        )
    return in_maps, bases


def _combine(results, bases, Wp, bp):
    U_full = np.zeros((NBAGS + 128, D), np.float64)
    den_full = np.zeros(NBAGS + 128, np.float64)
    for c in range(N_CORES):
        u = results[c]["u"]
        U_full[bases[c] : bases[c] + 128] += u[:, :D]
        den_full[bases[c] : bases[c] + 128] += u[:, D]
    U_full = U_full[:NBAGS]
    den_full = den_full[:NBAGS]
    den_safe = np.where(den_full == 0, 1.0, den_full)
    M = (U_full / den_safe[:, None]).astype(np.float32)
    proj = (M @ np.asarray(Wp, np.float32) + np.asarray(bp, np.float32)).astype(
        np.float32
    )
    nrm = np.maximum(np.linalg.norm(proj, axis=1, keepdims=True), 1e-12)
    proj = (proj / nrm).astype(np.float32)
    return M, proj


def _numpy_fallback(x, idxs, W1, b1, Wt, bt, Ws, bs, Wa, ba, Wp, bp):
    H = np.maximum(x @ W1 + b1, 0.0).astype(np.float32)
    At = np.tanh(H @ Wt + bt)
    As = 1.0 / (1.0 + np.exp(-(H @ Ws + bs)))
    a = ((At * As) @ Wa)[:, 0] + np.asarray(ba).reshape(-1)[0]
    a = a - a.max()
    e = np.exp(a)
    den = np.zeros(NBAGS)
    np.add.at(den, idxs, e)
    U = np.zeros((NBAGS, D))
    np.add.at(U, idxs, e[:, None] * H)
    den = np.where(den == 0, 1.0, den)
    M = (U / den[:, None]).astype(np.float32)
    proj = (M @ Wp + bp).astype(np.float32)
    nrm = np.maximum(np.linalg.norm(proj, axis=1, keepdims=True), 1e-12)
    return M, (proj / nrm).astype(np.float32)


def kernel(x, idxs, W1, b1, Wt, bt, Ws, bs, Wa, ba, Wp, bp):
    from concourse.bass_utils import run_bass_kernel_spmd

    x = np.ascontiguousarray(np.asarray(x), np.float32)
    idxs = np.asarray(idxs).astype(np.int64)
    args = [np.asarray(v, np.float32) for v in (W1, b1, Wt, bt, Ws, bs, Wa, ba)]
    W1, b1, Wt, bt, Ws, bs, Wa, ba = args
    Wp = np.asarray(Wp, np.float32)
    bp = np.asarray(bp, np.float32)

    in_maps, bases = _host_prep(x, idxs, W1, b1, Wt, bt, Ws, bs, Wa, ba)
    if in_maps is None:
        return _numpy_fallback(
            x, idxs, W1, b1, Wt, bt, Ws, bs, Wa, ba, Wp, bp
        )
    nc = _get_nc(x.shape[0] // N_CORES)
    res = run_bass_kernel_spmd(nc, in_maps, list(range(N_CORES)), trace=False)
    return _combine(res.results, bases, Wp, bp)
